# revision 12
# baseline (speedup 1.0000x reference)
"""Trainium2 Bass kernel for nn_Block_22823456211331 (dense transformer block).

Sharding: DP2 x TP4 over 8 NeuronCores.
  core c: batch b = c // 4, tp rank t = c % 4, heads 4t..4t+3.
  - RMSNorm folded: w_ln1/w_ln2 into weight matrices (host), per-row scale r
    folded into rope cos/sin (q,k) / applied per-partition (v) / applied in GLU.
  - Attention computed with transposed scores sT[k,q] per 512-wide q-chunk;
    softmax denominator via an extra ones-column appended to V.
  - Attention output projection produces a per-core partial [S,E]; one
    ReduceScatter over each 4-core group yields each core's 512-row slice.
  - FFN (GLU) is row-sharded: each core runs the full FFN on its 512 rows
    (full W1/W2 streamed), so no second collective is needed.
All matmuls in bf16 (fp32 PSUM accumulation); residuals/norms in fp32.
"""
import os
import sys
import numpy as np

sys.path.insert(0, "/opt/trn_rl_repo")

import ml_dtypes

B, S, E, H, HS = 2, 2048, 1024, 16, 64
TP = 4
HPC = H // TP          # 4 heads per core
SL = S // TP           # 512-row slice per core
EPS = 1e-6
BF = ml_dtypes.bfloat16

_CACHE = {}
LAST_RESULTS = None


# ----------------------------------------------------------------- host prep

def _prep_core(inputs, c):
    x = np.asarray(inputs["x"], np.float32)
    w_ln1 = np.asarray(inputs["w_ln1"], np.float32)
    w_ln2 = np.asarray(inputs["w_ln2"], np.float32)
    Wq = np.asarray(inputs["Wq"], np.float32)
    Wk = np.asarray(inputs["Wk"], np.float32)
    Wv = np.asarray(inputs["Wv"], np.float32)
    Wo = np.asarray(inputs["Wo"], np.float32)
    W1 = np.asarray(inputs["W1"], np.float32)
    W2 = np.asarray(inputs["W2"], np.float32)

    perm = np.concatenate([np.arange(0, HS, 2), np.arange(1, HS, 2)])
    b = c // TP
    t = c % TP
    heads = list(range(HPC * t, HPC * t + HPC))
    xb = x[b]
    xT = np.ascontiguousarray(xb.T).astype(BF).reshape(8, 128, S)
    x_slice = np.ascontiguousarray(xb[SL * t:SL * (t + 1)])
    Wq_c = np.concatenate([(w_ln1[:, None] * Wq[h])[:, perm] for h in heads], axis=1)
    Wk_c = np.concatenate([(w_ln1[:, None] * Wk[h])[:, perm] for h in heads], axis=1)
    Wv_c = np.concatenate([(w_ln1[:, None] * Wv[h]) for h in heads], axis=1)
    wq_t = np.ascontiguousarray(
        Wq_c.astype(BF).reshape(8, 128, 2, 128).transpose(1, 2, 0, 3)
        .reshape(128, 2048))
    wk_t = np.ascontiguousarray(
        Wk_c.astype(BF).reshape(8, 128, 2, 128).transpose(1, 2, 0, 3)
        .reshape(128, 2048))
    wv_t = np.ascontiguousarray(
        Wv_c.astype(BF).reshape(8, 128, 256).transpose(1, 0, 2)
        .reshape(128, 2048))
    Wo_c = Wo[heads[0] * HS:(heads[-1] + 1) * HS]
    wo_t = np.ascontiguousarray(
        Wo_c.astype(BF).reshape(2, 128, 2, 512).transpose(1, 0, 2, 3)
        .reshape(128, 2048))
    W1p = w_ln2[:, None] * W1
    w1_t = np.ascontiguousarray(
        W1p.astype(BF).reshape(8, 128, 64, 128).transpose(2, 1, 0, 3)
        .reshape(64, 128, 1024))
    w2_t = np.ascontiguousarray(
        W2.astype(BF).reshape(32, 128, 2, 512).transpose(2, 0, 1, 3))
    return dict(xT=xT, xs=x_slice, wq=wq_t, wk=wk_t, wv=wv_t, wo=wo_t,
                w1=w1_t, w2=w2_t)


def _prep_shared(inputs):
    cos = np.asarray(inputs["cos"], np.float32)
    sin = np.asarray(inputs["sin"], np.float32)
    b1 = np.asarray(inputs["b1"], np.float32)
    bo = np.asarray(inputs["bo"], np.float32)
    b2 = np.asarray(inputs["b2"], np.float32)
    sh = dict(
        cosT=np.ascontiguousarray(np.tile(cos.T, (4, 1))),
        sinT=np.ascontiguousarray(np.tile(sin.T, (4, 1))),
        b1c=np.ascontiguousarray(b1.reshape(64, 128).T),
        bo_r=bo.reshape(1, E).copy(),
        b2_r=b2.reshape(1, E).copy(),
        eye_bf=np.eye(128, dtype=BF),
        eye_f=np.eye(128, dtype=np.float32),
        ones_col_bf=np.ones((128, 1), dtype=BF),
        ones_row_f=np.ones((1, 128), dtype=np.float32),
        mask=np.where(np.arange(128)[:, None] > np.arange(128)[None, :],
                      -1e9, 0.0).astype(np.float32),
    )
    return sh


# -------------------------------------------------------------- graph build

def build_graph():
    from concourse import bass, bacc, mybir, tile
    from contextlib import ExitStack

    f32 = mybir.dt.float32
    bf = mybir.dt.bfloat16
    AF = mybir.ActivationFunctionType
    OP = mybir.AluOpType

    nc = bacc.Bacc("TRN2", target_bir_lowering=False, debug=False, num_devices=8)

    xT_e = nc.declare_dram_parameter("xT", [8, 128, S], bf, isOutput=False)
    xs_e = nc.declare_dram_parameter("xs", [SL, E], f32, isOutput=False)
    wq_e = nc.declare_dram_parameter("wq", [128, 2048], bf, isOutput=False)
    wk_e = nc.declare_dram_parameter("wk", [128, 2048], bf, isOutput=False)
    wv_e = nc.declare_dram_parameter("wv", [128, 2048], bf, isOutput=False)
    wo_e = nc.declare_dram_parameter("wo", [128, 2048], bf, isOutput=False)
    w1_e = nc.declare_dram_parameter("w1", [64, 128, 1024], bf, isOutput=False)
    w2_e = nc.declare_dram_parameter("w2", [2, 32, 128, 512], bf, isOutput=False)
    cos_e = nc.declare_dram_parameter("cosT", [128, S], f32, isOutput=False)
    sin_e = nc.declare_dram_parameter("sinT", [128, S], f32, isOutput=False)
    b1c_e = nc.declare_dram_parameter("b1c", [128, 64], f32, isOutput=False)
    bo_e = nc.declare_dram_parameter("bo_r", [1, E], f32, isOutput=False)
    b2_e = nc.declare_dram_parameter("b2_r", [1, E], f32, isOutput=False)
    eyebf_e = nc.declare_dram_parameter("eye_bf", [128, 128], bf, isOutput=False)
    eyef_e = nc.declare_dram_parameter("eye_f", [128, 128], f32, isOutput=False)
    onc_e = nc.declare_dram_parameter("ones_col_bf", [128, 1], bf, isOutput=False)
    onr_e = nc.declare_dram_parameter("ones_row_f", [1, 128], f32, isOutput=False)
    mask_e = nc.declare_dram_parameter("mask", [128, 128], f32, isOutput=False)
    out_e = nc.declare_dram_parameter("out", [SL, E], f32, isOutput=True)

    MM = nc.tensor.matmul
    ACT = nc.scalar
    DVE = nc.vector
    DMA = nc.sync.dma_start

    def T(pool, shape, dtype, name, bufs=None):
        kw = {} if bufs is None else {"bufs": bufs}
        return pool.tile(shape, dtype, name=name, tag=name, **kw)

    with tile.TileContext(nc) as tc, ExitStack() as top:
        dram = top.enter_context(tc.tile_pool(name="dram", bufs=1, space="DRAM"))
        partials = [T(dram, [S, 512], bf, f"partial{ec}") for ec in range(2)]
        rs_outs = [T(dram, [SL, 512], bf, f"rs_out{ec}") for ec in range(2)]

        cpool = top.enter_context(tc.tile_pool(name="const", bufs=1))
        eye_bf = T(cpool, [128, 128], bf, "eye_bf_t")
        DMA(out=eye_bf[:], in_=eyebf_e[:])
        eye_f = T(cpool, [128, 128], f32, "eye_f_t")
        DMA(out=eye_f[:], in_=eyef_e[:])
        ones_col = T(cpool, [128, 1], bf, "ones_col_t")
        DMA(out=ones_col[:], in_=onc_e[:])
        ones_row = T(cpool, [1, 128], f32, "ones_row_t")
        DMA(out=ones_row[:], in_=onr_e[:])
        mask = T(cpool, [128, 128], f32, "mask_t")
        DMA(out=mask[:], in_=mask_e[:])
        b1c = T(cpool, [128, 64], f32, "b1c_t")
        DMA(out=b1c[:], in_=b1c_e[:])
        bo_row = T(cpool, [1, E], f32, "bo_row")
        DMA(out=bo_row[:], in_=bo_e[:])
        b2row = T(cpool, [1, E], f32, "b2row")
        DMA(out=b2row[:], in_=b2_e[:])

        ppool = top.enter_context(tc.tile_pool(name="pers", bufs=1))
        # q/k head-pair tiles [128, S]: head h lives at rows 64*(h%2) of
        # tile h//2 (keeps matmul base partitions in {0, 64})
        q2 = [T(ppool, [128, S], bf, f"q2_{hp}") for hp in range(2)]
        k2 = [T(ppool, [128, S], bf, f"k2_{hp}") for hp in range(2)]
        v4 = [T(ppool, [128, 4 * 65], bf, f"v4_{st}") for st in range(16)]


        def hq(h):
            return q2[h // 2][64 * (h % 2):64 * (h % 2) + 64, :]

        def hk(h):
            return k2[h // 2][64 * (h % 2):64 * (h % 2) + 64, :]

        # ---------------- phase A-early: r1, QKV, rope --------------------
        with tc.tile_pool(name="early", bufs=1) as epool, \
             tc.tile_pool(name="etmp", bufs=2) as etmp, \
             tc.tile_pool(name="eps", bufs=1, space="PSUM") as eps, \
             tc.tile_pool(name="eps2", bufs=2, space="PSUM") as eps2:
            xT = []
            for et in range(8):
                xt = T(epool, [128, S], bf, f"xT{et}")
                DMA(out=xt[:], in_=xT_e[et])
                xT.append(xt)
            wq_sb = T(epool, [128, 16 * 128], bf, "wq_sb")
            DMA(out=wq_sb[:], in_=wq_e[:])
            wk_sb = T(epool, [128, 16 * 128], bf, "wk_sb")
            DMA(out=wk_sb[:], in_=wk_e[:])
            wv_sb = T(epool, [128, 8 * 256], bf, "wv_sb")
            DMA(out=wv_sb[:], in_=wv_e[:])
            cosT = T(epool, [128, S], f32, "cosT_t")
            DMA(out=cosT[:], in_=cos_e[:])
            sinT = T(epool, [128, S], f32, "sinT_t")
            DMA(out=sinT[:], in_=sin_e[:])

            # r1 = rsqrt(mean(x^2)+eps) over E, from xT squares
            r1f = T(epool, [1, S], f32, "r1f")
            for sc in range(4):
                r1ps = eps2.tile([1, 512], f32, tag="r1ps", name="r1ps")
                for et in range(8):
                    xsq = etmp.tile([128, 512], bf, tag="xsq", name="xsq",
                                    bufs=3)
                    DVE.tensor_tensor(xsq[:], xT[et][:, 512 * sc:512 * (sc + 1)],
                                      xT[et][:, 512 * sc:512 * (sc + 1)], OP.mult)
                    MM(r1ps[:], ones_col[:], xsq[:],
                       start=(et == 0), stop=(et == 7))
                ACT.activation(r1f[0:1, 512 * sc:512 * (sc + 1)], r1ps[:],
                               AF.Copy, bias=EPS, scale=1.0 / E)

            # r1 broadcast [128, S] and C/S tiles (pattern tiled 4x so any
            # 32-row slice carries the full cos/sin*r1 pattern)
            r1b = T(epool, [128, S], f32, "r1b")
            for sc in range(4):
                bps = eps2.tile([128, 512], f32, tag="r1b_ps", name="bps", bufs=1)
                MM(bps[:], ones_row[0:1, 0:128], r1f[0:1, 512 * sc:512 * (sc + 1)],
                   start=True, stop=True)
                rtmp = etmp.tile([128, 512], f32, tag="rtmp", name="rtmp")
                DVE.reciprocal(rtmp[:], bps[:])
                ACT.activation(r1b[:, 512 * sc:512 * (sc + 1)], rtmp[:], AF.Sqrt)
            C2k = T(epool, [128, S], bf, "C2k")
            S2k = T(epool, [128, S], bf, "S2k")
            C2q = T(epool, [128, S], bf, "C2q")
            S2q = T(epool, [128, S], bf, "S2q")
            DVE.tensor_tensor(C2k[:], cosT[:], r1b[:], OP.mult)
            DVE.tensor_tensor(S2k[:], sinT[:], r1b[:], OP.mult)
            DVE.tensor_scalar(C2q[:], C2k[:], 0.125, None, OP.mult)
            DVE.tensor_scalar(S2q[:], S2k[:], 0.125, None, OP.mult)

            # r1 as per-partition columns [128, 16]
            r1cp = T(eps, [128, 16], f32, "r1cp")
            for i in range(16):
                MM(r1cp[:, i:i + 1], r1b[0:1, 128 * i:128 * (i + 1)],
                   ones_row[0:1, 0:1], start=True, stop=True)
            r1c = T(epool, [128, 16], f32, "r1c")
            ACT.activation(r1c[:], r1cp[:], AF.Copy)

            # QKV projections
            for (wsb, dst) in ((wq_sb, q2), (wk_sb, k2)):
                for hp in range(2):
                    for sc in range(4):
                        qps = eps2.tile([128, 512], f32, tag="qkv", name="qps")
                        for et in range(8):
                            MM(qps[:],
                               wsb[:, (hp * 8 + et) * 128:(hp * 8 + et + 1) * 128],
                               xT[et][:, 512 * sc:512 * (sc + 1)],
                               start=(et == 0), stop=(et == 7))
                        ACT.activation(dst[hp][:, 512 * sc:512 * (sc + 1)],
                                       qps[:], AF.Copy)
            for st in range(16):
                vps = eps2.tile([128, 256], f32, tag="v", name="vps")
                for et in range(8):
                    MM(vps[:], xT[et][:, 128 * st:128 * (st + 1)],
                       wv_sb[:, 256 * et:256 * (et + 1)],
                       start=(et == 0), stop=(et == 7))
                DVE.memset(v4[st][:], 1.0)
                for h in range(4):
                    DVE.tensor_scalar(v4[st][:, 65 * h:65 * h + 64],
                                      vps[:, 64 * h:64 * h + 64],
                                      r1c[:, st:st + 1], None, OP.mult)

            # rope (in place over q2/k2 head slices); all tensor_tensor
            # INPUT pairs share a base partition (HW verifier requirement)
            for h in range(4):
                hp, hh = h // 2, h % 2
                a0, b0 = 64 * hh, 64 * hh + 32
                for (tile2, Ct, St) in ((q2[hp], C2q, S2q), (k2[hp], C2k, S2k)):
                    A = tile2[a0:a0 + 32, :]
                    Bm = tile2[b0:b0 + 32, :]
                    t1 = etmp.tile([128, S], bf, tag="t1", name="t1")
                    t2 = etmp.tile([128, S], bf, tag="t2", name="t2")
                    # products for A' at rows a0, for B' at rows b0
                    DVE.tensor_tensor(t1[a0:a0 + 32, :], A,
                                      Ct[a0:a0 + 32, :], OP.mult)
                    DVE.tensor_tensor(t2[a0:a0 + 32, :], Bm,
                                      St[b0:b0 + 32, :], OP.mult)
                    DVE.tensor_tensor(t2[b0:b0 + 32, :], A,
                                      St[a0:a0 + 32, :], OP.mult)
                    DVE.tensor_tensor(t1[b0:b0 + 32, :], Bm,
                                      Ct[b0:b0 + 32, :], OP.mult)
                    DVE.tensor_tensor(A, t1[a0:a0 + 32, :],
                                      t2[a0:a0 + 32, :], OP.subtract)
                    DVE.tensor_tensor(Bm, t2[b0:b0 + 32, :],
                                      t1[b0:b0 + 32, :], OP.add)

        # ---------------- phase A-attn: scores/softmax/PV, out-proj -------
        # qc-outer: each 512-wide q-chunk is scored, softmaxed, PV'd for all
        # heads, normalized, projected, and DMA'd; the two column-halves of
        # the partial go to two independent bf16 ReduceScatters so the
        # second one overlaps early FFN work.
        with tc.tile_pool(name="attn", bufs=1) as apool, \
             tc.tile_pool(name="ptp", bufs=2) as ptp, \
             tc.tile_pool(name="aps1", bufs=1, space="PSUM") as aps:
            wo_sb = T(apool, [128, 4 * 512], bf, "wo_sb")
            DMA(out=wo_sb[:], in_=wo_e[:])
            o4T = [T(apool, [128, S], bf, f"o4T{i}") for i in range(2)]
            sums_h = [T(apool, [1, S], f32, f"sums_h{h}") for h in range(4)]
            for qc in range(4):
                ktmax = 4 * (qc + 1)
                q0 = 512 * qc
                for h in range(4):
                    pts = [ptp.tile([128, 512], bf, tag=f"pt{kt}", name=f"pt{kt}")
                           for kt in range(ktmax)]
                    for kt in range(ktmax):
                        sps = aps.tile([128, 512], f32, tag="s", name="sps",
                                       bufs=2)
                        MM(sps[:], hk(h)[:, 128 * kt:128 * (kt + 1)],
                           hq(h)[:, q0:q0 + 512], start=True, stop=True)
                        j = kt - 4 * qc
                        if j >= 0:
                            DVE.tensor_tensor(sps[:, 128 * j:128 * (j + 1)],
                                              sps[:, 128 * j:128 * (j + 1)],
                                              mask[:], OP.add)
                        ACT.activation(pts[kt][:], sps[:], AF.Exp)
                        if j > 0:
                            DVE.memset(pts[kt][:, 0:128 * j], 0.0)
                    ops = [aps.tile([65, 128], f32, tag=f"o{qtl}",
                                    name=f"ops{qtl}", bufs=1)
                           for qtl in range(4)]
                    for kt in range(ktmax):
                        for qtl in range(4):
                            MM(ops[qtl][:], v4[kt][:, 65 * h:65 * (h + 1)],
                               pts[kt][:, 128 * qtl:128 * (qtl + 1)],
                               start=(kt == 0), stop=(kt == ktmax - 1))
                    for qtl in range(4):
                        qt = 4 * qc + qtl
                        ACT.activation(
                            sums_h[h][0:1, 128 * qt:128 * (qt + 1)],
                            ops[qtl][64:65, :], AF.Copy)
                        ACT.activation(
                            o4T[h // 2][64 * (h % 2):64 * (h % 2) + 64,
                                        128 * qt:128 * (qt + 1)],
                            ops[qtl][0:64, :], AF.Copy)
                # normalize this q-chunk of o4T (reciprocal on 128 partitions)
                for i in range(2):
                    ips = aps.tile([128, 512], f32, tag="opA", name="ips")
                    MM(ips[0:64, :], ones_row[0:1, 0:64],
                       sums_h[2 * i][0:1, q0:q0 + 512], start=True, stop=True)
                    MM(ips[64:128, :], ones_row[0:1, 0:64],
                       sums_h[2 * i + 1][0:1, q0:q0 + 512],
                       start=True, stop=True)
                    ibf = ptp.tile([128, 512], f32, tag="ibf", name="ibf")
                    DVE.reciprocal(ibf[:], ips[:])
                    ib = ptp.tile([128, 512], bf, tag="invb", name="ib")
                    ACT.activation(ib[:], ibf[:], AF.Copy)
                    DVE.tensor_tensor(o4T[i][:, q0:q0 + 512],
                                      o4T[i][:, q0:q0 + 512], ib[:], OP.mult)
                # out-proj for this q-chunk's 4 row-tiles, ec-major
                for ec in range(2):
                    for st in range(4 * qc, 4 * qc + 4):
                        pps = aps.tile([128, 512], f32,
                                       tag=("opA" if ec == 0 else "opB"),
                                       name="pps")
                        for hd in range(2):
                            MM(pps[:], o4T[hd][:, 128 * st:128 * (st + 1)],
                               wo_sb[:, (hd * 2 + ec) * 512:
                                     (hd * 2 + ec + 1) * 512],
                               start=(hd == 0), stop=(hd == 1))
                        pcp = ptp.tile([128, 512], bf, tag="pcp", name="pcp",
                                       bufs=3)
                        if ec == 0:
                            ACT.activation(pcp[:], pps[:], AF.Copy)
                        else:
                            DVE.tensor_copy(pcp[:], pps[:])
                        DMA(out=partials[ec][128 * st:128 * (st + 1), :],
                            in_=pcp[:])

        # ---------------- collectives: 2 column-split bf16 RS --------------
        for ec in range(2):
            nc.gpsimd.collective_compute(
                "ReduceScatter", mybir.AluOpType.add,
                replica_groups=[[0, 1, 2, 3], [4, 5, 6, 7]],
                ins=[partials[ec].opt()], outs=[rs_outs[ec].opt()])

        # ---------------- phase B: x_mid, FFN ------------------------------
        with tc.tile_pool(name="ffn", bufs=1) as fpool, \
             tc.tile_pool(name="ftmp", bufs=2) as ftmp:
            xm2 = [T(fpool, [128, E], f32, f"xm2_{st}") for st in range(4)]
            xmT = [T(fpool, [128, 512], bf, f"xmT{et}") for et in range(8)]
            acch = T(fpool, [128, 8], f32, "acch")
            m4 = T(fpool, [128, 4], f32, "m4")
            i4 = T(fpool, [128, 4], f32, "i4")
            r2c4 = T(fpool, [128, 4], f32, "r2c4")

            with tc.tile_pool(name="b1pool", bufs=1) as bpool, \
                 tc.tile_pool(name="b1ps", bufs=2, space="PSUM") as bps_pool:
                bo_b = T(bpool, [128, E], f32, "bo_b")
                b2b = T(bpool, [128, E], f32, "b2b")
                for (srow, dstt) in ((bo_row, bo_b), (b2row, b2b)):
                    for ec in range(2):
                        bps = bps_pool.tile([128, 512], f32, tag="bias",
                                            name="bps")
                        MM(bps[:], ones_row[0:1, :],
                           srow[0:1, 512 * ec:512 * (ec + 1)],
                           start=True, stop=True)
                        ACT.activation(dstt[:, 512 * ec:512 * (ec + 1)],
                                       bps[:], AF.Copy)
                for st in range(4):
                    xs_sb = ftmp.tile([128, E], f32, tag="xs", name="xs_sb")
                    DMA(out=xs_sb[:], in_=xs_e[128 * st:128 * (st + 1), :])
                    xmid = ftmp.tile([128, E], f32, tag="xmid", name="xmid")
                    xmb = ftmp.tile([128, E], bf, tag="xmb", name="xmb")
                    for ch in range(2):
                        c0 = 512 * ch
                        rs_sb = ftmp.tile([128, 512], bf, tag=f"rs{ch}",
                                          name="rs_sb", bufs=2)
                        DMA(out=rs_sb[:],
                            in_=rs_outs[ch][128 * st:128 * (st + 1), :])
                        rs_f = ftmp.tile([128, 512], f32, tag=f"rsf{ch}",
                                         name="rs_f", bufs=2)
                        ACT.activation(rs_f[:], rs_sb[:], AF.Copy)
                        DVE.tensor_tensor(xmid[:, c0:c0 + 512], rs_f[:],
                                          xs_sb[:, c0:c0 + 512], OP.add)
                        DVE.tensor_tensor(xmid[:, c0:c0 + 512],
                                          xmid[:, c0:c0 + 512],
                                          bo_b[:, c0:c0 + 512], OP.add)
                        scr = ftmp.tile([128, 512], bf, tag="scr", name="scr")
                        ACT.activation(scr[:], xmid[:, c0:c0 + 512], AF.Square,
                                       accum_out=acch[:, 2 * st + ch:
                                                      2 * st + ch + 1])
                        DVE.tensor_tensor(xm2[st][:, c0:c0 + 512],
                                          xmid[:, c0:c0 + 512],
                                          b2b[:, c0:c0 + 512], OP.add)
                        ACT.activation(xmb[:, c0:c0 + 512],
                                       xmid[:, c0:c0 + 512], AF.Copy)
                        for et in range(4 * ch, 4 * ch + 4):
                            tps = bps_pool.tile([128, 128], f32, tag="tp",
                                                name="tps")
                            MM(tps[:], xmb[:, 128 * et:128 * (et + 1)],
                               eye_bf[:], start=True, stop=True)
                            ACT.activation(xmT[et][:, 128 * st:128 * (st + 1)],
                                           tps[:], AF.Copy)
                    DVE.tensor_tensor(m4[:, st:st + 1],
                                      acch[:, 2 * st:2 * st + 1],
                                      acch[:, 2 * st + 1:2 * st + 2], OP.add)
                    ACT.activation(m4[:, st:st + 1], m4[:, st:st + 1],
                                   AF.Copy, bias=EPS, scale=1.0 / E)
                    DVE.reciprocal(i4[:, st:st + 1], m4[:, st:st + 1])
                    ACT.activation(r2c4[:, st:st + 1], i4[:, st:st + 1],
                                   AF.Sqrt)
                # r2 row + broadcast [128, 512]
                r2rp = bps_pool.tile([1, 512], f32, tag="r2r", name="r2rp",
                                     bufs=1)
                for st in range(4):
                    MM(r2rp[0:1, 128 * st:128 * (st + 1)], r2c4[:, st:st + 1],
                       eye_f[:], start=True, stop=True)
                r2row = T(fpool, [1, 512], f32, "r2row")
                ACT.activation(r2row[:], r2rp[:], AF.Copy)
                r2bp = bps_pool.tile([128, 512], f32, tag="r2b", name="r2bp",
                                     bufs=1)
                MM(r2bp[:], ones_row[0:1, :], r2row[0:1, :],
                   start=True, stop=True)
                r2b = T(fpool, [128, 512], f32, "r2b")
                ACT.activation(r2b[:], r2bp[:], AF.Copy)

            # FFN1 + GLU -> fT tiles
            fT = [T(fpool, [128, 512], bf, f"fT{cc}") for cc in range(32)]
            with tc.tile_pool(name="f1ps", bufs=2, space="PSUM") as f1ps:
                for cc in range(32):
                    w1a = ftmp.tile([128, 8 * 128], bf, tag="w1a", name="w1a",
                                    bufs=3)
                    DMA(out=w1a[:], in_=w1_e[cc])
                    w1g = ftmp.tile([128, 8 * 128], bf, tag="w1g", name="w1g",
                                    bufs=3)
                    DMA(out=w1g[:], in_=w1_e[cc + 32])
                    aps_ = f1ps.tile([128, 512], f32, tag="fa", name="aps_")
                    gps = f1ps.tile([128, 512], f32, tag="fg", name="gps")
                    for et in range(8):
                        MM(aps_[:], w1a[:, 128 * et:128 * (et + 1)], xmT[et][:],
                           start=(et == 0), stop=(et == 7))
                        MM(gps[:], w1g[:, 128 * et:128 * (et + 1)], xmT[et][:],
                           start=(et == 0), stop=(et == 7))
                    m1 = ftmp.tile([128, 512], f32, tag="m1", name="m1")
                    DVE.tensor_tensor(m1[:], gps[:], r2b[:], OP.mult)
                    sig = ftmp.tile([128, 512], f32, tag="sig", name="sig")
                    ACT.activation(sig[:], m1[:], AF.Sigmoid,
                                   bias=b1c[:, cc + 32:cc + 33])
                    tt_ = ftmp.tile([128, 512], f32, tag="tt", name="tt_")
                    DVE.tensor_tensor(tt_[:], aps_[:], r2b[:], OP.mult)
                    DVE.scalar_tensor_tensor(fT[cc][:], tt_[:],
                                             b1c[:, cc:cc + 1], sig[:],
                                             OP.add, OP.mult)

            # FFN2: stream W2 once (ht-outer), 8 live PSUM accumulators
            with tc.tile_pool(name="f2ps", bufs=1, space="PSUM") as f2ps:
                yps = [[f2ps.tile([128, 512], f32, tag=f"y{st}_{ec}",
                                  name=f"yps{st}_{ec}") for ec in range(2)]
                       for st in range(4)]
                for ht in range(32):
                    w2a = ftmp.tile([128, 512], bf, tag="w2a", name="w2a",
                                    bufs=3)
                    DMA(out=w2a[:], in_=w2_e[0, ht])
                    w2g = ftmp.tile([128, 512], bf, tag="w2g", name="w2g",
                                    bufs=3)
                    DMA(out=w2g[:], in_=w2_e[1, ht])
                    for st in range(4):
                        MM(yps[st][0][:], fT[ht][:, 128 * st:128 * (st + 1)],
                           w2a[:], start=(ht == 0), stop=(ht == 31))
                        MM(yps[st][1][:], fT[ht][:, 128 * st:128 * (st + 1)],
                           w2g[:], start=(ht == 0), stop=(ht == 31))
                for st in range(4):
                    for ec in range(2):
                        osb = ftmp.tile([128, 512], f32, tag="osb", name="osb")
                        DVE.tensor_tensor(osb[:], yps[st][ec][:],
                                          xm2[st][:, 512 * ec:512 * (ec + 1)],
                                          OP.add)
                        DMA(out=out_e[128 * st:128 * (st + 1),
                                      512 * ec:512 * (ec + 1)], in_=osb[:])

    nc.compile()
    return nc


# ------------------------------------------------------------------ driver

def kernel(**inputs):
    global LAST_RESULTS
    from concourse.bass_utils import run_bass_kernel_spmd

    if "nc" not in _CACHE:
        _CACHE["nc"] = build_graph()
    nc = _CACHE["nc"]

    sh = _prep_shared(inputs)
    in_maps = []
    for c in range(8):
        m = dict(_prep_core(inputs, c))
        m.update(sh)
        in_maps.append(m)

    res = run_bass_kernel_spmd(nc, in_maps, core_ids=list(range(8)))
    LAST_RESULTS = res
    out = np.zeros((B, S, E), np.float32)
    for c in range(8):
        b, t = c // TP, c % TP
        out[b, SL * t:SL * (t + 1), :] = res.results[c]["out"]
    return out


if __name__ == "__main__":
    nc = build_graph()
    print("graph built + compiled OK")


# revision 13
# speedup vs baseline: 1.2753x; 1.2753x over previous
"""Trainium2 Bass kernel for nn_Block_22823456211331 (dense transformer block).

Sharding: DP2 x TP4 over 8 NeuronCores.
  core c: batch b = c // 4, tp rank t = c % 4, heads 4t..4t+3.
  - RMSNorm folded: w_ln1/w_ln2 into weight matrices (host), per-row scale r
    folded into rope cos/sin (q,k) / applied per-partition (v) / applied in GLU.
  - Attention computed with transposed scores sT[k,q] per 512-wide q-chunk;
    softmax denominator via an extra ones-column appended to V.
  - Attention output projection produces a per-core partial [S,E]; one
    ReduceScatter over each 4-core group yields each core's 512-row slice.
  - FFN (GLU) is row-sharded: each core runs the full FFN on its 512 rows
    (full W1/W2 streamed), so no second collective is needed.
All matmuls in bf16 (fp32 PSUM accumulation); residuals/norms in fp32.
"""
import os
import sys
import numpy as np

sys.path.insert(0, "/opt/trn_rl_repo")

import ml_dtypes

B, S, E, H, HS = 2, 2048, 1024, 16, 64
TP = 4
HPC = H // TP          # 4 heads per core
SL = S // TP           # 512-row slice per core
EPS = 1e-6
BF = ml_dtypes.bfloat16

_CACHE = {}
LAST_RESULTS = None


# ----------------------------------------------------------------- host prep

def _prep_core(inputs, c):
    x = np.asarray(inputs["x"], np.float32)
    w_ln1 = np.asarray(inputs["w_ln1"], np.float32)
    w_ln2 = np.asarray(inputs["w_ln2"], np.float32)
    Wq = np.asarray(inputs["Wq"], np.float32)
    Wk = np.asarray(inputs["Wk"], np.float32)
    Wv = np.asarray(inputs["Wv"], np.float32)
    Wo = np.asarray(inputs["Wo"], np.float32)
    W1 = np.asarray(inputs["W1"], np.float32)
    W2 = np.asarray(inputs["W2"], np.float32)

    perm = np.concatenate([np.arange(0, HS, 2), np.arange(1, HS, 2)])
    b = c // TP
    t = c % TP
    heads = list(range(HPC * t, HPC * t + HPC))
    xb = x[b]
    xT = np.ascontiguousarray(xb.T).astype(BF).reshape(8, 128, S)
    x_slice = np.ascontiguousarray(np.concatenate(
        [xb[256 * t:256 * (t + 1)], xb[1024 + 256 * t:1024 + 256 * (t + 1)]]))
    Wq_c = np.concatenate([(w_ln1[:, None] * Wq[h])[:, perm] for h in heads], axis=1)
    Wk_c = np.concatenate([(w_ln1[:, None] * Wk[h])[:, perm] for h in heads], axis=1)
    Wv_c = np.concatenate([(w_ln1[:, None] * Wv[h]) for h in heads], axis=1)
    wq_t = np.ascontiguousarray(
        Wq_c.astype(BF).reshape(8, 128, 2, 128).transpose(1, 2, 0, 3)
        .reshape(128, 2048))
    wk_t = np.ascontiguousarray(
        Wk_c.astype(BF).reshape(8, 128, 2, 128).transpose(1, 2, 0, 3)
        .reshape(128, 2048))
    wv_t = np.ascontiguousarray(
        Wv_c.astype(BF).reshape(8, 128, 256).transpose(1, 0, 2)
        .reshape(128, 2048))
    Wo_c = Wo[heads[0] * HS:(heads[-1] + 1) * HS]
    wo_t = np.ascontiguousarray(
        Wo_c.astype(BF).reshape(2, 128, 2, 512).transpose(1, 0, 2, 3)
        .reshape(128, 2048))
    W1p = w_ln2[:, None] * W1
    w1_t = np.ascontiguousarray(
        W1p.astype(BF).reshape(8, 128, 64, 128).transpose(2, 1, 0, 3)
        .reshape(64, 128, 1024))
    w2_t = np.ascontiguousarray(
        W2.astype(BF).reshape(32, 128, 2, 512).transpose(2, 0, 1, 3))
    return dict(xT=xT, xs=x_slice, wq=wq_t, wk=wk_t, wv=wv_t, wo=wo_t,
                w1=w1_t, w2=w2_t)


def _prep_shared(inputs):
    cos = np.asarray(inputs["cos"], np.float32)
    sin = np.asarray(inputs["sin"], np.float32)
    b1 = np.asarray(inputs["b1"], np.float32)
    bo = np.asarray(inputs["bo"], np.float32)
    b2 = np.asarray(inputs["b2"], np.float32)
    sh = dict(
        cosT=np.ascontiguousarray(np.tile(cos.T, (4, 1))),
        sinT=np.ascontiguousarray(np.tile(sin.T, (4, 1))),
        b1c=np.ascontiguousarray(b1.reshape(64, 128).T),
        bo_r=bo.reshape(1, E).copy(),
        b2_r=b2.reshape(1, E).copy(),
        eye_bf=np.eye(128, dtype=BF),
        eye_f=np.eye(128, dtype=np.float32),
        ones_col_bf=np.ones((128, 1), dtype=BF),
        ones_row_f=np.ones((1, 128), dtype=np.float32),
        mask=np.where(np.arange(128)[:, None] > np.arange(128)[None, :],
                      -1e9, 0.0).astype(np.float32),
    )
    return sh


# -------------------------------------------------------------- graph build

def build_graph():
    from concourse import bass, bacc, mybir, tile
    from contextlib import ExitStack

    f32 = mybir.dt.float32
    bf = mybir.dt.bfloat16
    AF = mybir.ActivationFunctionType
    OP = mybir.AluOpType

    nc = bacc.Bacc("TRN2", target_bir_lowering=False, debug=False, num_devices=8)

    xT_e = nc.declare_dram_parameter("xT", [8, 128, S], bf, isOutput=False)
    xs_e = nc.declare_dram_parameter("xs", [SL, E], f32, isOutput=False)
    wq_e = nc.declare_dram_parameter("wq", [128, 2048], bf, isOutput=False)
    wk_e = nc.declare_dram_parameter("wk", [128, 2048], bf, isOutput=False)
    wv_e = nc.declare_dram_parameter("wv", [128, 2048], bf, isOutput=False)
    wo_e = nc.declare_dram_parameter("wo", [128, 2048], bf, isOutput=False)
    w1_e = nc.declare_dram_parameter("w1", [64, 128, 1024], bf, isOutput=False)
    w2_e = nc.declare_dram_parameter("w2", [2, 32, 128, 512], bf, isOutput=False)
    cos_e = nc.declare_dram_parameter("cosT", [128, S], f32, isOutput=False)
    sin_e = nc.declare_dram_parameter("sinT", [128, S], f32, isOutput=False)
    b1c_e = nc.declare_dram_parameter("b1c", [128, 64], f32, isOutput=False)
    bo_e = nc.declare_dram_parameter("bo_r", [1, E], f32, isOutput=False)
    b2_e = nc.declare_dram_parameter("b2_r", [1, E], f32, isOutput=False)
    eyebf_e = nc.declare_dram_parameter("eye_bf", [128, 128], bf, isOutput=False)
    eyef_e = nc.declare_dram_parameter("eye_f", [128, 128], f32, isOutput=False)
    onc_e = nc.declare_dram_parameter("ones_col_bf", [128, 1], bf, isOutput=False)
    onr_e = nc.declare_dram_parameter("ones_row_f", [1, 128], f32, isOutput=False)
    mask_e = nc.declare_dram_parameter("mask", [128, 128], f32, isOutput=False)
    out_e = nc.declare_dram_parameter("out", [SL, E], f32, isOutput=True)

    MM = nc.tensor.matmul
    ACT = nc.scalar
    DVE = nc.vector
    DMA = nc.sync.dma_start

    def T(pool, shape, dtype, name, bufs=None):
        kw = {} if bufs is None else {"bufs": bufs}
        return pool.tile(shape, dtype, name=name, tag=name, **kw)

    with tile.TileContext(nc) as tc, ExitStack() as top:
        dram = top.enter_context(tc.tile_pool(name="dram", bufs=1, space="DRAM"))
        partials = [T(dram, [1024, E], bf, f"partial{rh}") for rh in range(2)]
        rs_outs = [T(dram, [256, E], bf, f"rs_out{rh}") for rh in range(2)]

        cpool = top.enter_context(tc.tile_pool(name="const", bufs=1))
        eye_bf = T(cpool, [128, 128], bf, "eye_bf_t")
        DMA(out=eye_bf[:], in_=eyebf_e[:])
        eye_f = T(cpool, [128, 128], f32, "eye_f_t")
        DMA(out=eye_f[:], in_=eyef_e[:])
        ones_col = T(cpool, [128, 1], bf, "ones_col_t")
        DMA(out=ones_col[:], in_=onc_e[:])
        ones_row = T(cpool, [1, 128], f32, "ones_row_t")
        DMA(out=ones_row[:], in_=onr_e[:])
        mask = T(cpool, [128, 128], f32, "mask_t")
        DMA(out=mask[:], in_=mask_e[:])
        b1c = T(cpool, [128, 64], f32, "b1c_t")
        DMA(out=b1c[:], in_=b1c_e[:])
        bo_row = T(cpool, [1, E], f32, "bo_row")
        DMA(out=bo_row[:], in_=bo_e[:])
        b2row = T(cpool, [1, E], f32, "b2row")
        DMA(out=b2row[:], in_=b2_e[:])

        ppool = top.enter_context(tc.tile_pool(name="pers", bufs=1))
        # q/k head-pair tiles [128, S]: head h lives at rows 64*(h%2) of
        # tile h//2 (keeps matmul base partitions in {0, 64})
        q2 = [T(ppool, [128, S], bf, f"q2_{hp}") for hp in range(2)]
        k2 = [T(ppool, [128, S], bf, f"k2_{hp}") for hp in range(2)]
        v4 = [T(ppool, [128, 4 * 65], bf, f"v4_{st}") for st in range(16)]


        def hq(h):
            return q2[h // 2][64 * (h % 2):64 * (h % 2) + 64, :]

        def hk(h):
            return k2[h // 2][64 * (h % 2):64 * (h % 2) + 64, :]

        # ---------------- phase A-early: r1, QKV, rope --------------------
        with tc.tile_pool(name="early", bufs=1) as epool, \
             tc.tile_pool(name="etmp", bufs=2) as etmp, \
             tc.tile_pool(name="eps", bufs=1, space="PSUM") as eps, \
             tc.tile_pool(name="eps2", bufs=2, space="PSUM") as eps2:
            xT = []
            for et in range(8):
                xt = T(epool, [128, S], bf, f"xT{et}")
                DMA(out=xt[:], in_=xT_e[et])
                xT.append(xt)
            wq_sb = T(epool, [128, 16 * 128], bf, "wq_sb")
            DMA(out=wq_sb[:], in_=wq_e[:])
            wk_sb = T(epool, [128, 16 * 128], bf, "wk_sb")
            DMA(out=wk_sb[:], in_=wk_e[:])
            wv_sb = T(epool, [128, 8 * 256], bf, "wv_sb")
            DMA(out=wv_sb[:], in_=wv_e[:])
            cosT = T(epool, [128, S], f32, "cosT_t")
            DMA(out=cosT[:], in_=cos_e[:])
            sinT = T(epool, [128, S], f32, "sinT_t")
            DMA(out=sinT[:], in_=sin_e[:])

            # r1 = rsqrt(mean(x^2)+eps) over E, from xT squares
            r1f = T(epool, [1, S], f32, "r1f")
            for sc in range(4):
                r1ps = eps2.tile([1, 512], f32, tag="r1ps", name="r1ps")
                for et in range(8):
                    xsq = etmp.tile([128, 512], bf, tag="xsq", name="xsq",
                                    bufs=3)
                    DVE.tensor_tensor(xsq[:], xT[et][:, 512 * sc:512 * (sc + 1)],
                                      xT[et][:, 512 * sc:512 * (sc + 1)], OP.mult)
                    MM(r1ps[:], ones_col[:], xsq[:],
                       start=(et == 0), stop=(et == 7))
                ACT.activation(r1f[0:1, 512 * sc:512 * (sc + 1)], r1ps[:],
                               AF.Copy, bias=EPS, scale=1.0 / E)

            # r1 broadcast [128, S] and C/S tiles (pattern tiled 4x so any
            # 32-row slice carries the full cos/sin*r1 pattern)
            r1b = T(epool, [128, S], f32, "r1b")
            for sc in range(4):
                bps = eps2.tile([128, 512], f32, tag="r1b_ps", name="bps", bufs=1)
                MM(bps[:], ones_row[0:1, 0:128], r1f[0:1, 512 * sc:512 * (sc + 1)],
                   start=True, stop=True)
                rtmp = etmp.tile([128, 512], f32, tag="rtmp", name="rtmp")
                DVE.reciprocal(rtmp[:], bps[:])
                ACT.activation(r1b[:, 512 * sc:512 * (sc + 1)], rtmp[:], AF.Sqrt)
            C2k = T(epool, [128, S], bf, "C2k")
            S2k = T(epool, [128, S], bf, "S2k")
            C2q = T(epool, [128, S], bf, "C2q")
            S2q = T(epool, [128, S], bf, "S2q")
            DVE.tensor_tensor(C2k[:], cosT[:], r1b[:], OP.mult)
            DVE.tensor_tensor(S2k[:], sinT[:], r1b[:], OP.mult)
            DVE.tensor_scalar(C2q[:], C2k[:], 0.125, None, OP.mult)
            DVE.tensor_scalar(S2q[:], S2k[:], 0.125, None, OP.mult)

            # r1 as per-partition columns [128, 16]
            r1cp = T(eps, [128, 16], f32, "r1cp")
            for i in range(16):
                MM(r1cp[:, i:i + 1], r1b[0:1, 128 * i:128 * (i + 1)],
                   ones_row[0:1, 0:1], start=True, stop=True)
            r1c = T(epool, [128, 16], f32, "r1c")
            ACT.activation(r1c[:], r1cp[:], AF.Copy)

            # QKV projections
            for (wsb, dst) in ((wq_sb, q2), (wk_sb, k2)):
                for hp in range(2):
                    for sc in range(4):
                        qps = eps2.tile([128, 512], f32, tag="qkv", name="qps")
                        for et in range(8):
                            MM(qps[:],
                               wsb[:, (hp * 8 + et) * 128:(hp * 8 + et + 1) * 128],
                               xT[et][:, 512 * sc:512 * (sc + 1)],
                               start=(et == 0), stop=(et == 7))
                        ACT.activation(dst[hp][:, 512 * sc:512 * (sc + 1)],
                                       qps[:], AF.Copy)
            for st in range(16):
                vps = eps2.tile([128, 256], f32, tag="v", name="vps")
                for et in range(8):
                    MM(vps[:], xT[et][:, 128 * st:128 * (st + 1)],
                       wv_sb[:, 256 * et:256 * (et + 1)],
                       start=(et == 0), stop=(et == 7))
                DVE.memset(v4[st][:], 1.0)
                for h in range(4):
                    DVE.tensor_scalar(v4[st][:, 65 * h:65 * h + 64],
                                      vps[:, 64 * h:64 * h + 64],
                                      r1c[:, st:st + 1], None, OP.mult)

            # rope (in place over q2/k2 head slices); all tensor_tensor
            # INPUT pairs share a base partition (HW verifier requirement)
            for h in range(4):
                hp, hh = h // 2, h % 2
                a0, b0 = 64 * hh, 64 * hh + 32
                for (tile2, Ct, St) in ((q2[hp], C2q, S2q), (k2[hp], C2k, S2k)):
                    A = tile2[a0:a0 + 32, :]
                    Bm = tile2[b0:b0 + 32, :]
                    t1 = etmp.tile([128, S], bf, tag="t1", name="t1")
                    t2 = etmp.tile([128, S], bf, tag="t2", name="t2")
                    # products for A' at rows a0, for B' at rows b0
                    DVE.tensor_tensor(t1[a0:a0 + 32, :], A,
                                      Ct[a0:a0 + 32, :], OP.mult)
                    DVE.tensor_tensor(t2[a0:a0 + 32, :], Bm,
                                      St[b0:b0 + 32, :], OP.mult)
                    DVE.tensor_tensor(t2[b0:b0 + 32, :], A,
                                      St[a0:a0 + 32, :], OP.mult)
                    DVE.tensor_tensor(t1[b0:b0 + 32, :], Bm,
                                      Ct[b0:b0 + 32, :], OP.mult)
                    DVE.tensor_tensor(A, t1[a0:a0 + 32, :],
                                      t2[a0:a0 + 32, :], OP.subtract)
                    DVE.tensor_tensor(Bm, t2[b0:b0 + 32, :],
                                      t1[b0:b0 + 32, :], OP.add)

        # ---------------- phase A-attn: scores/softmax/PV, out-proj -------
        # qc-outer: each 512-wide q-chunk is scored, softmaxed, PV'd for all
        # heads, normalized, projected, and DMA'd; the two column-halves of
        # the partial go to two independent bf16 ReduceScatters so the
        # second one overlaps early FFN work.
        with tc.tile_pool(name="attn", bufs=1) as apool, \
             tc.tile_pool(name="ptp", bufs=2) as ptp, \
             tc.tile_pool(name="aps1", bufs=1, space="PSUM") as aps:
            wo_sb = T(apool, [128, 4 * 512], bf, "wo_sb")
            DMA(out=wo_sb[:], in_=wo_e[:])
            o4T = [T(apool, [128, S], bf, f"o4T{i}") for i in range(2)]
            sums_h = [T(apool, [1, S], f32, f"sums_h{h}") for h in range(4)]
            for qc in range(4):
                ktmax = 4 * (qc + 1)
                q0 = 512 * qc
                for h in range(4):
                    pts = [ptp.tile([128, 512], bf, tag=f"pt{kt}", name=f"pt{kt}")
                           for kt in range(ktmax)]
                    for kt in range(ktmax):
                        j = kt - 4 * qc
                        w0 = 128 * j if j > 0 else 0  # fully-masked col prefix
                        sps = aps.tile([128, 512], f32, tag="s", name="sps",
                                       bufs=2)
                        MM(sps[:, w0:512], hk(h)[:, 128 * kt:128 * (kt + 1)],
                           hq(h)[:, q0 + w0:q0 + 512], start=True, stop=True)
                        if j >= 0:
                            DVE.tensor_tensor(sps[:, 128 * j:128 * (j + 1)],
                                              sps[:, 128 * j:128 * (j + 1)],
                                              mask[:], OP.add)
                        ACT.activation(pts[kt][:, w0:512], sps[:, w0:512],
                                       AF.Exp)
                    ops = [aps.tile([65, 128], f32, tag=f"o{qtl}",
                                    name=f"ops{qtl}", bufs=1)
                           for qtl in range(4)]
                    for qtl in range(4):
                        kthi = 4 * qc + qtl + 1  # k-tiles kt >= kthi are all-0
                        for kt in range(kthi):
                            MM(ops[qtl][:], v4[kt][:, 65 * h:65 * (h + 1)],
                               pts[kt][:, 128 * qtl:128 * (qtl + 1)],
                               start=(kt == 0), stop=(kt == kthi - 1))
                    for qtl in range(4):
                        qt = 4 * qc + qtl
                        ACT.activation(
                            sums_h[h][0:1, 128 * qt:128 * (qt + 1)],
                            ops[qtl][64:65, :], AF.Copy)
                        ACT.activation(
                            o4T[h // 2][64 * (h % 2):64 * (h % 2) + 64,
                                        128 * qt:128 * (qt + 1)],
                            ops[qtl][0:64, :], AF.Copy)
                # normalize this q-chunk of o4T (reciprocal on 128 partitions)
                for i in range(2):
                    ips = aps.tile([128, 512], f32, tag="opA", name="ips")
                    MM(ips[0:64, :], ones_row[0:1, 0:64],
                       sums_h[2 * i][0:1, q0:q0 + 512], start=True, stop=True)
                    MM(ips[64:128, :], ones_row[0:1, 0:64],
                       sums_h[2 * i + 1][0:1, q0:q0 + 512],
                       start=True, stop=True)
                    ibf = ptp.tile([128, 512], f32, tag="ibf", name="ibf")
                    DVE.reciprocal(ibf[:], ips[:])
                    ib = ptp.tile([128, 512], bf, tag="invb", name="ib")
                    ACT.activation(ib[:], ibf[:], AF.Copy)
                    DVE.tensor_tensor(o4T[i][:, q0:q0 + 512],
                                      o4T[i][:, q0:q0 + 512], ib[:], OP.mult)
                # out-proj for this q-chunk's 4 row-tiles
                for st in range(4 * qc, 4 * qc + 4):
                    pcp = ptp.tile([128, E], bf, tag="pcp", name="pcp",
                                   bufs=3)
                    for ec in range(2):
                        pps = aps.tile([128, 512], f32,
                                       tag=("opA" if ec == 0 else "opB"),
                                       name="pps")
                        for hd in range(2):
                            MM(pps[:], o4T[hd][:, 128 * st:128 * (st + 1)],
                               wo_sb[:, (hd * 2 + ec) * 512:
                                     (hd * 2 + ec + 1) * 512],
                               start=(hd == 0), stop=(hd == 1))
                        if ec == 0:
                            ACT.activation(pcp[:, 0:512], pps[:], AF.Copy)
                        else:
                            DVE.tensor_copy(pcp[:, 512:1024], pps[:])
                    rh, strow = st // 8, (st % 8) * 128
                    DMA(out=partials[rh][strow:strow + 128, :], in_=pcp[:])
                if qc == 1:
                    # rows 0:1024 fully projected -> overlap RS with qc 2,3
                    nc.gpsimd.collective_compute(
                        "ReduceScatter", mybir.AluOpType.add,
                        replica_groups=[[0, 1, 2, 3], [4, 5, 6, 7]],
                        ins=[partials[0].opt()], outs=[rs_outs[0].opt()])
            nc.gpsimd.collective_compute(
                "ReduceScatter", mybir.AluOpType.add,
                replica_groups=[[0, 1, 2, 3], [4, 5, 6, 7]],
                ins=[partials[1].opt()], outs=[rs_outs[1].opt()])

        # ---------------- phase B: x_mid, FFN ------------------------------
        with tc.tile_pool(name="ffn", bufs=1) as fpool, \
             tc.tile_pool(name="ftmp", bufs=2) as ftmp:
            xm2 = [T(fpool, [128, E], f32, f"xm2_{st}") for st in range(4)]
            xmT = [T(fpool, [128, 512], bf, f"xmT{et}") for et in range(8)]
            acch = T(fpool, [128, 8], f32, "acch")
            m4 = T(fpool, [128, 4], f32, "m4")
            i4 = T(fpool, [128, 4], f32, "i4")
            r2c4 = T(fpool, [128, 4], f32, "r2c4")

            with tc.tile_pool(name="b1pool", bufs=1) as bpool, \
                 tc.tile_pool(name="b1ps", bufs=2, space="PSUM") as bps_pool:
                bo_b = T(bpool, [128, E], f32, "bo_b")
                b2b = T(bpool, [128, E], f32, "b2b")
                for (srow, dstt) in ((bo_row, bo_b), (b2row, b2b)):
                    for ec in range(2):
                        bps = bps_pool.tile([128, 512], f32, tag="bias",
                                            name="bps")
                        MM(bps[:], ones_row[0:1, :],
                           srow[0:1, 512 * ec:512 * (ec + 1)],
                           start=True, stop=True)
                        ACT.activation(dstt[:, 512 * ec:512 * (ec + 1)],
                                       bps[:], AF.Copy)
                for st in range(4):
                    xs_sb = ftmp.tile([128, E], f32, tag="xs", name="xs_sb")
                    DMA(out=xs_sb[:], in_=xs_e[128 * st:128 * (st + 1), :])
                    xmid = ftmp.tile([128, E], f32, tag="xmid", name="xmid")
                    xmb = ftmp.tile([128, E], bf, tag="xmb", name="xmb")
                    rs_sb = ftmp.tile([128, E], bf, tag="rs", name="rs_sb",
                                      bufs=2)
                    DMA(out=rs_sb[:],
                        in_=rs_outs[st // 2][128 * (st % 2):
                                             128 * (st % 2) + 128, :])
                    for ch in range(2):
                        c0 = 512 * ch
                        rs_f = ftmp.tile([128, 512], f32, tag=f"rsf{ch}",
                                         name="rs_f", bufs=2)
                        ACT.activation(rs_f[:], rs_sb[:, c0:c0 + 512], AF.Copy)
                        DVE.tensor_tensor(xmid[:, c0:c0 + 512], rs_f[:],
                                          xs_sb[:, c0:c0 + 512], OP.add)
                        DVE.tensor_tensor(xmid[:, c0:c0 + 512],
                                          xmid[:, c0:c0 + 512],
                                          bo_b[:, c0:c0 + 512], OP.add)
                        scr = ftmp.tile([128, 512], bf, tag="scr", name="scr")
                        ACT.activation(scr[:], xmid[:, c0:c0 + 512], AF.Square,
                                       accum_out=acch[:, 2 * st + ch:
                                                      2 * st + ch + 1])
                        DVE.tensor_tensor(xm2[st][:, c0:c0 + 512],
                                          xmid[:, c0:c0 + 512],
                                          b2b[:, c0:c0 + 512], OP.add)
                        ACT.activation(xmb[:, c0:c0 + 512],
                                       xmid[:, c0:c0 + 512], AF.Copy)
                        for et in range(4 * ch, 4 * ch + 4):
                            tps = bps_pool.tile([128, 128], f32, tag="tp",
                                                name="tps")
                            MM(tps[:], xmb[:, 128 * et:128 * (et + 1)],
                               eye_bf[:], start=True, stop=True)
                            ACT.activation(xmT[et][:, 128 * st:128 * (st + 1)],
                                           tps[:], AF.Copy)
                    DVE.tensor_tensor(m4[:, st:st + 1],
                                      acch[:, 2 * st:2 * st + 1],
                                      acch[:, 2 * st + 1:2 * st + 2], OP.add)
                    ACT.activation(m4[:, st:st + 1], m4[:, st:st + 1],
                                   AF.Copy, bias=EPS, scale=1.0 / E)
                    DVE.reciprocal(i4[:, st:st + 1], m4[:, st:st + 1])
                    ACT.activation(r2c4[:, st:st + 1], i4[:, st:st + 1],
                                   AF.Sqrt)
                # r2 row + broadcast [128, 512]
                r2rp = bps_pool.tile([1, 512], f32, tag="r2r", name="r2rp",
                                     bufs=1)
                for st in range(4):
                    MM(r2rp[0:1, 128 * st:128 * (st + 1)], r2c4[:, st:st + 1],
                       eye_f[:], start=True, stop=True)
                r2row = T(fpool, [1, 512], f32, "r2row")
                ACT.activation(r2row[:], r2rp[:], AF.Copy)
                r2bp = bps_pool.tile([128, 512], f32, tag="r2b", name="r2bp",
                                     bufs=1)
                MM(r2bp[:], ones_row[0:1, :], r2row[0:1, :],
                   start=True, stop=True)
                r2b = T(fpool, [128, 512], f32, "r2b")
                ACT.activation(r2b[:], r2bp[:], AF.Copy)

            # FFN1 + GLU -> fT tiles
            fT = [T(fpool, [128, 512], bf, f"fT{cc}") for cc in range(32)]
            with tc.tile_pool(name="f1ps", bufs=2, space="PSUM") as f1ps:
                for cc in range(32):
                    w1a = ftmp.tile([128, 8 * 128], bf, tag="w1a", name="w1a",
                                    bufs=3)
                    DMA(out=w1a[:], in_=w1_e[cc])
                    w1g = ftmp.tile([128, 8 * 128], bf, tag="w1g", name="w1g",
                                    bufs=3)
                    DMA(out=w1g[:], in_=w1_e[cc + 32])
                    aps_ = f1ps.tile([128, 512], f32, tag="fa", name="aps_")
                    gps = f1ps.tile([128, 512], f32, tag="fg", name="gps")
                    for et in range(8):
                        MM(aps_[:], w1a[:, 128 * et:128 * (et + 1)], xmT[et][:],
                           start=(et == 0), stop=(et == 7))
                        MM(gps[:], w1g[:, 128 * et:128 * (et + 1)], xmT[et][:],
                           start=(et == 0), stop=(et == 7))
                    m1 = ftmp.tile([128, 512], f32, tag="m1", name="m1")
                    DVE.tensor_tensor(m1[:], gps[:], r2b[:], OP.mult)
                    sig = ftmp.tile([128, 512], f32, tag="sig", name="sig")
                    ACT.activation(sig[:], m1[:], AF.Sigmoid,
                                   bias=b1c[:, cc + 32:cc + 33])
                    tt_ = ftmp.tile([128, 512], f32, tag="tt", name="tt_")
                    DVE.tensor_tensor(tt_[:], aps_[:], r2b[:], OP.mult)
                    DVE.scalar_tensor_tensor(fT[cc][:], tt_[:],
                                             b1c[:, cc:cc + 1], sig[:],
                                             OP.add, OP.mult)

            # FFN2: stream W2 once (ht-outer), 8 live PSUM accumulators
            with tc.tile_pool(name="f2ps", bufs=1, space="PSUM") as f2ps:
                yps = [[f2ps.tile([128, 512], f32, tag=f"y{st}_{ec}",
                                  name=f"yps{st}_{ec}") for ec in range(2)]
                       for st in range(4)]
                for ht in range(32):
                    w2a = ftmp.tile([128, 512], bf, tag="w2a", name="w2a",
                                    bufs=3)
                    DMA(out=w2a[:], in_=w2_e[0, ht])
                    w2g = ftmp.tile([128, 512], bf, tag="w2g", name="w2g",
                                    bufs=3)
                    DMA(out=w2g[:], in_=w2_e[1, ht])
                    for st in range(4):
                        MM(yps[st][0][:], fT[ht][:, 128 * st:128 * (st + 1)],
                           w2a[:], start=(ht == 0), stop=(ht == 31))
                        MM(yps[st][1][:], fT[ht][:, 128 * st:128 * (st + 1)],
                           w2g[:], start=(ht == 0), stop=(ht == 31))
                for st in range(4):
                    for ec in range(2):
                        osb = ftmp.tile([128, 512], f32, tag="osb", name="osb")
                        DVE.tensor_tensor(osb[:], yps[st][ec][:],
                                          xm2[st][:, 512 * ec:512 * (ec + 1)],
                                          OP.add)
                        DMA(out=out_e[128 * st:128 * (st + 1),
                                      512 * ec:512 * (ec + 1)], in_=osb[:])

    nc.compile()
    return nc


# ------------------------------------------------------------------ driver

def kernel(**inputs):
    global LAST_RESULTS
    from concourse.bass_utils import run_bass_kernel_spmd

    if "nc" not in _CACHE:
        _CACHE["nc"] = build_graph()
    nc = _CACHE["nc"]

    sh = _prep_shared(inputs)
    in_maps = []
    for c in range(8):
        m = dict(_prep_core(inputs, c))
        m.update(sh)
        in_maps.append(m)

    res = run_bass_kernel_spmd(nc, in_maps, core_ids=list(range(8)))
    LAST_RESULTS = res
    out = np.zeros((B, S, E), np.float32)
    for c in range(8):
        b, t = c // TP, c % TP
        o = res.results[c]["out"]
        out[b, 256 * t:256 * (t + 1), :] = o[:256]
        out[b, 1024 + 256 * t:1024 + 256 * (t + 1), :] = o[256:]
    return out


if __name__ == "__main__":
    nc = build_graph()
    print("graph built + compiled OK")
